# revision 1
# baseline (speedup 1.0000x reference)
"""Multi-head attention (AttnProcessor2_0) on 8 TRN2 NeuronCores.

Problem: B=2, S=4096, C=640, H=10, Dh=64.
  q/k/v = hs @ W{q,k,v}.T ; per-head scores = q k^T / 8 ; softmax ;
  out = probs v ; y = out @ Wo.T + b_out + hs

Sharding (no collectives): core c -> batch b=c//4, query block g=c%4
(1024 queries).  Each core recomputes full K/V for its batch (head-dim
on partitions), computes its own S/4 x S attention block, output
projection, bias+residual.  Host passes hidden states TRANSPOSED and
ROLLED by the query offset so the same SPMD program works on every
core (softmax+PV are permutation-invariant along the key axis).

Device layout (everything feature-on-partition, token-on-free):
  kT [640, 4096] (5 chunks of 128 = 2 heads each)  "scoresT" = K Q^T
  qT [128, 1024] per head pair (rows 0:64 head even, 64:128 head odd)
  v  [4096, 650] (65-stride per head: 64 cols + ones col -> softmax
     denominators fall out of the PV matmul as PSUM row 64)
  QK: both heads of a pair run CONCURRENTLY as K=64 row-tiled matmuls
     (tile_position (0,0) and (64,0)) writing adjacent PSUM banks --
     2x the padded-contraction QK throughput of the old layout.
  probs: scoresT in PSUM -> ScalarE exp -> bf16 SBUF
  normalization: pv [65,512] copied to SBUF right after the PV
     accumulation stops (frees the PSUM bank), reciprocal of denom row,
     rank-1 PE outer product into a scratch PSUM bank to broadcast
     across partitions, DVE mult.
All matmuls bf16 (f32 PSUM accumulation).
"""

import sys

if "/opt/trn_rl_repo" not in sys.path:
    sys.path.insert(0, "/opt/trn_rl_repo")

from collections import deque
from contextlib import ExitStack

import ml_dtypes
import numpy as np

import concourse.bass as bass
import concourse.tile as tile
from concourse import mybir
from concourse.bass import ts

BF16 = mybir.dt.bfloat16
F32 = mybir.dt.float32
F8 = mybir.dt.float8e4

B, S, C = 2, 4096, 640
H, DH = 10, 64
NCORES = 8
GROUP = 4  # cores per batch element
SQ = S // GROUP  # 1024 queries per core
SCALE = 0.125  # 1/sqrt(64)
CCH = C // 128  # 5 feature chunks (2 heads each)
NJT = S // 512  # 8 key tiles for K proj
NJC = S // 128  # 32 key chunks for attention
NIT = SQ // 512  # 2 query tiles
VST = DH + 1  # 65: per-head stride in v tiles (ones col appended)

# Schraudolph exp offload: selected score chunks compute exp on the DVE
# as a bf16 bit-trick (one tensor_scalar: bits = round(s*A + B) viewed as
# bf16 gives 2^(s*log2e) with ~2% per-element jitter and ~zero mean; any
# constant bias cancels in the softmax ratio).  This moves work off the
# bottleneck ScalarE onto the (slack) DVE.  Set empty to disable.
SCHRAUD_A = SCALE * 128.0 / float(np.log(2.0))
SCHRAUD_B = 127.0 * 128.0 - 7.45
OFFLOAD_JC = frozenset()


def build_nc() -> bass.Bass:
    nc = bass.Bass()
    hsT = nc.declare_dram_parameter("hsT", [C, S], BF16, isOutput=False)
    res = nc.declare_dram_parameter("res", [C, SQ], F32, isOutput=False)
    wqT = nc.declare_dram_parameter("wqT", [C, C], BF16, isOutput=False)
    wkT = nc.declare_dram_parameter("wkT", [C, C], BF16, isOutput=False)
    wvT = nc.declare_dram_parameter("wvT", [C, C], BF16, isOutput=False)
    woT = nc.declare_dram_parameter("woT", [C, C], BF16, isOutput=False)
    out = nc.declare_dram_parameter("out", [C, SQ], F32, isOutput=True)

    with ExitStack() as ctx:
        tc = ctx.enter_context(tile.TileContext(nc))
        # outer pool: tensors whose lifetime spans projections AND attention
        sb = ctx.enter_context(tc.tile_pool(name="sb", bufs=1))

        kT_sb = [sb.tile([128, S], BF16, tag=f"kT{i}", name=f"kT{i}") for i in range(CCH)]
        # head-pair q: rows 0:64 = even head, 64:128 = odd head.  The QK
        # matmuls are K=64 row-tiled (tile_position (0,0)/(64,0)) and run
        # concurrently in the PE array -- no zero padding needed.
        qT_sb = [sb.tile([128, SQ], BF16, tag=f"qT{i}", name=f"qT{i}") for i in range(CCH)]
        v_sb = [sb.tile([128, H * VST], BF16, tag=f"v{j}", name=f"v{j}") for j in range(NJC)]
        ones_sb = sb.tile([128, DH], BF16, tag="ones", name="ones")
        nc.vector.memset(ones_sb[:], 1.0)

        # prefetch the exp table set while DMAs stream (the pseudo
        # ACT_TABLE_LOAD walrus inserts before the first real exp would
        # otherwise land on the critical path, ~1.3us)
        warm = sb.tile([1, 16], F32, tag="warm", name="warm")
        nc.vector.memset(warm[:], 0.0)
        nc.scalar.activation(warm[:], warm[:],
                             mybir.ActivationFunctionType.Exp,
                             bias=0.0, scale=0.0)

        # ---------------- load + first projections ----------------
        # Each input tensor is ONE wide SBUF tile filled by ONE DMA (the
        # Sync engine issues triggers at ~600ns each -- 20 small DMAs
        # serialized the old startup).  Chunk cc of a tensor lives at
        # free-offset cc*width; h3/wk3/... are [128, chunk, width] views.
        load = ctx.enter_context(tc.tile_pool(name="load", bufs=1))
        hsT_big = load.tile([128, CCH * S], BF16, tag="hsT", name="hsT")
        h3 = hsT_big[:].rearrange("p (f s) -> p f s", s=S)
        wk3 = load.tile([128, CCH * C], BF16, tag="wk", name="wk")[:] \
            .rearrange("p (f c) -> p f c", c=C)
        wq3 = load.tile([128, CCH * C], BF16, tag="wq", name="wq")[:] \
            .rearrange("p (f c) -> p f c", c=C)
        wv3 = load.tile([128, CCH * C], BF16, tag="wv", name="wv")[:] \
            .rearrange("p (f c) -> p f c", c=C)
        # full Wo resident (800KB): kills the per-oproj weight DMAs and
        # zero-padding; with head-paired attn the contraction is all-real
        wo3 = load.tile([128, CCH * C], BF16, tag="wo", name="wo")[:] \
            .rearrange("p (f c) -> p f c", c=C)
        nc.sync.dma_start(wk3, wkT[:, :].rearrange("(f p) c -> p f c", p=128))
        nc.sync.dma_start(
            h3[:, :, 0:SQ],
            hsT[:, 0:SQ].rearrange("(f p) s -> p f s", p=128),
        )
        nc.sync.dma_start(wq3, wqT[:, :].rearrange("(f p) c -> p f c", p=128))
        nc.sync.dma_start(wv3, wvT[:, :].rearrange("(f p) c -> p f c", p=128))
        nc.sync.dma_start(wo3, woT[:, :].rearrange("(f p) c -> p f c", p=128))

        def emit_hsT_tail():
            # deferred until after the first exp so ScalarE's conservative
            # vector-clock waits don't cover this 4MB of DMA
            for blk in range(SQ, S, SQ):
                nc.sync.dma_start(
                    h3[:, :, blk : blk + SQ],
                    hsT[:, blk : blk + SQ].rearrange("(f p) s -> p f s", p=128),
                )

        def proj_ops(w3, dst, dc, jt, pool):
            # one K/Q projection stripe as 6 micro-ops (5 MMs + cast) so
            # the background drain never inserts more than ~2 matmuls
            # between attention-stream matmuls (a whole 5-MM burst would
            # stall the exp pipeline ~600ns per burst)
            st = {}

            def mm(cc):
                def f():
                    if "ps" not in st:
                        st["ps"] = pool.tile([128, 512], F32, tag="pp",
                                             name="pp", bufs=2)
                    nc.tensor.matmul(
                        st["ps"][:],
                        w3[:, cc, ts(dc, 128)],
                        h3[:, cc, ts(jt, 512)],
                        start=(cc == 0),
                        stop=(cc == CCH - 1),
                    )
                return f

            def cast():
                nc.vector.tensor_copy(dst[:, ts(jt, 512)], st["ps"][:])

            return [mm(cc) for cc in range(CCH)] + [cast]

        def emit_kproj(dc, jt, pool):
            for f in proj_ops(wk3, kT_sb[dc], dc, jt, pool):
                f()

        def emit_qproj(dc, it, pool):
            for f in proj_ops(wq3, qT_sb[dc], dc, it, pool):
                f()

        def emit_vproj(jc, pool):
            vt = v_sb[jc]
            v3 = vt[:].rearrange("p (h x) -> p h x", x=VST)
            for d0, dn in ((0, 512), (512, 128)):
                ps = pool.tile([128, 512], F32, tag="pp", name="pp", bufs=2)
                for cc in range(CCH):
                    nc.tensor.matmul(
                        ps[:, 0:dn],
                        h3[:, cc, ts(jc, 128)],
                        wv3[:, cc, d0 : d0 + dn],
                        start=(cc == 0),
                        stop=(cc == CCH - 1),
                    )
                nc.vector.tensor_copy(
                    v3[:, d0 // DH : (d0 + dn) // DH, 0:DH],
                    ps[:, 0:dn].rearrange("p (h x) -> p h x", x=DH),
                )

        # ones columns of all v tiles set once up front (DVE is idle
        # during the DMA-bound startup; doing this inside window 0 cost
        # ~0.7us of DVE per chunk right where the PE is most oversubscribed)
        for jc in range(NJC):
            v3c = v_sb[jc][:].rearrange("p (h x) -> p h x", x=VST)
            nc.vector.memset(v3c[:, :, DH : DH + 1], 1.0)

        with tc.tile_pool(name="pp0", bufs=2, space="PSUM") as pp0:
            for jt in range(2):
                emit_kproj(0, jt, pp0)
            emit_qproj(0, 0, pp0)

        # ---------------- attention phase ----------------
        # attn2[hp]: head pair packed (rows 0:64 even head, 64:128 odd) --
        # the output projection contracts all 128 rows with no padding
        attn2_sb = [sb.tile([128, SQ], BF16, tag=f"attn{p}", name=f"attn{p}")
                    for p in range(CCH)]
        with tc.tile_pool(name="ap", bufs=1, space="PSUM") as ap, \
             tc.tile_pool(name="pt", bufs=8) as pt_pool, \
             tc.tile_pool(name="ob", bufs=3) as ob, \
             tc.tile_pool(name="scratch", bufs=3) as scratch:
            def norm_dve(hp, pv0, pv1, p_isl, tail=False):
                # drain both pv accumulators into one packed tile (DVE
                # copies may shift partitions), denominators to rows 0/32
                # of a shared tile -> ONE reciprocal per head pair
                rawp = scratch.tile([128, 512], BF16, tag="raw", name="raw",
                                    bufs=2)
                nc.vector.tensor_copy(rawp[0:DH, :], pv0[0:DH, :])
                nc.vector.tensor_copy(rawp[DH:128, :], pv1[0:DH, :])
                rc = scratch.tile([33, 512], BF16, tag="rc", name="rc",
                                  bufs=2)
                dn = scratch.tile([33, 512], BF16, tag="dn", name="dn",
                                  bufs=2)
                nc.vector.tensor_copy(dn[0:1, :], pv0[DH : DH + 1, :])
                nc.vector.tensor_copy(dn[32:33, :], pv1[DH : DH + 1, :])
                with nc.allow_low_precision(reason="softmax recip bf16"):
                    if tail:
                        # tail: ScalarE is idle and the DVE reciprocal
                        # (3.3us, 8 cyc/elem iterative divide) would gate
                        # the final output projection; 1/x = exp(-ln(x))
                        # costs 2 short ACTs instead
                        lg = scratch.tile([33, 512], F32, tag="lg",
                                          name="lg", bufs=2)
                        nc.scalar.activation(
                            lg[:], dn[:], mybir.ActivationFunctionType.Ln,
                            bias=0.0, scale=1.0,
                        )
                        nc.scalar.activation(
                            rc[:], lg[:], mybir.ActivationFunctionType.Exp,
                            bias=0.0, scale=-1.0,
                        )
                    else:
                        nc.vector.reciprocal(rc[:], dn[:])
                return (hp, p_isl, rc, rawp)

            def norm_pe(state, anchor, pool):
                # two concurrent rank-1 PE broadcasts of the reciprocals
                # (row/col tiles (0,0) and (32,64)), pinned behind the
                # anchor QK so the slow DVE reciprocal is hidden
                hp, p_isl, rc, rawp = state
                ps = pool.tile([128, 512], F32, tag="pp", name="pp", bufs=2)
                r_mm = nc.tensor.matmul(
                    ps[0:DH, :],
                    ones_sb[0:1, 0:DH],
                    rc[0:1, :],
                    start=True,
                    stop=True,
                )
                nc.tensor.matmul(
                    ps[DH:128, :],
                    ones_sb[32:33, 0:DH],
                    rc[32:33, :],
                    start=True,
                    stop=True,
                )
                if anchor is not None:
                    tile.add_dep_helper(
                        r_mm.ins, anchor.ins, sync=False,
                        reason="norm R after anchor QK (hide recip latency)",
                    )
                nc.vector.tensor_mul(
                    attn2_sb[hp][:, p_isl], rawp[:], ps[:]
                )

            def oproj_ops(ec, it):
                st = {}

                def mm(r):
                    def f():
                        if "ps" not in st:
                            st["ps"] = ap.tile([128, 512], F32, tag="pp",
                                               name="pp", bufs=2)
                            rt = ob.tile([128, 512], F32, tag="rt",
                                         name="rt", bufs=2)
                            nc.sync.dma_start(
                                rt[:], res[ts(ec, 128), ts(it, 512)]
                            )
                            st["rt"] = rt
                        nc.tensor.matmul(
                            st["ps"][:],
                            wo3[:, r, ts(ec, 128)],
                            attn2_sb[r][:, ts(it, 512)],
                            start=(r == 0),
                            stop=(r == CCH - 1),
                        )
                    return f

                def fin():
                    ot = ob.tile([128, 512], F32, tag="ot", name="ot", bufs=2)
                    nc.vector.tensor_add(ot[:], st["ps"][:], st["rt"][:])
                    nc.sync.dma_start(out[ts(ec, 128), ts(it, 512)], ot[:])

                return [mm(r) for r in range(CCH)] + [fin]

            def emit_oproj(ec, it):
                for f in oproj_ops(ec, it):
                    f()

            # Background work flows through a micro-op queue drained at
            # most 2 ops per jc step, so no more than ~2 weight matmuls
            # ever sit between attention-stream matmuls on the PE (a
            # whole 5-MM projection burst stalls the exp pipeline).
            # Deadlines: a window's own kT stripes jt2..7 (first used at
            # jc8/12/../28) are enqueued at window start and finish by
            # ~jc17; the next window's kT jt0/jt1 + qT slice drain by the
            # window's end.  it1 windows carry the it0 output projections
            # and lazily-deferred it1 q-projections.
            bgq = deque()

            pending = []
            for it in range(NIT):
                isl = ts(it, 512)
                for hp in range(CCH):
                    h0, h1 = 2 * hp, 2 * hp + 1
                    if it == 0:
                        for jt in range(2, NJT):
                            bgq.extend(proj_ops(wk3, kT_sb[hp], hp, jt, ap))
                        if hp < CCH - 1:
                            for jt in range(2):
                                bgq.extend(
                                    proj_ops(wk3, kT_sb[hp + 1], hp + 1, jt, ap)
                                )
                            bgq.extend(proj_ops(wq3, qT_sb[hp + 1], hp + 1, 0, ap))
                        else:
                            bgq.extend(proj_ops(wq3, qT_sb[0], 0, 1, ap))
                    else:
                        if hp == 0:
                            bgq.extend(proj_ops(wq3, qT_sb[1], 1, 1, ap))
                        elif hp == 1:
                            bgq.extend(oproj_ops(0, 0))
                            bgq.extend(oproj_ops(1, 0))
                            bgq.extend(proj_ops(wq3, qT_sb[2], 2, 1, ap))
                        elif hp == 2:
                            bgq.extend(oproj_ops(2, 0))
                            bgq.extend(proj_ops(wq3, qT_sb[3], 3, 1, ap))
                        elif hp == 3:
                            bgq.extend(oproj_ops(3, 0))
                            bgq.extend(proj_ops(wq3, qT_sb[4], 4, 1, ap))
                        else:
                            bgq.extend(oproj_ops(4, 0))
                    vtodo = {}
                    if it == 0 and hp == 0:
                        # V chunks 0..7 front-loaded (keys 0:1024 resident
                        # before the hsT tail lands), then one chunk per
                        # step four steps ahead of its PV use
                        for jc in range(4):
                            vtodo[jc] = [2 * jc, 2 * jc + 1]
                        for jc in range(4, 28):
                            vtodo[jc] = [jc + 4]
                    pv0 = ap.tile([DH + 1, 512], F32, tag="pv0", bufs=1,
                                  name="pv0")
                    pv1 = ap.tile([DH + 1, 512], F32, tag="pv1", bufs=1,
                                  name="pv1")
                    for jc in range(NJC):
                        sc = ap.tile([128, 1024], F32, tag="sc", bufs=2,
                                     name="sc")
                        qk0 = nc.tensor.matmul(
                            sc[:, 0:512],
                            kT_sb[hp][0:DH, ts(jc, 128)],
                            qT_sb[hp][0:DH, isl],
                            start=True,
                            stop=True,
                        )
                        nc.tensor.matmul(
                            sc[:, 512:1024],
                            kT_sb[hp][DH:128, ts(jc, 128)],
                            qT_sb[hp][DH:128, isl],
                            start=True,
                            stop=True,
                        )
                        pt = pt_pool.tile([128, 1024], BF16,
                                          tag="pt", name="pt")
                        if (it, hp) != (0, 0) and jc in OFFLOAD_JC:
                            nc.vector.tensor_scalar(
                                out=pt[:].bitcast(mybir.dt.int16),
                                in0=sc[:],
                                scalar1=SCHRAUD_A,
                                scalar2=SCHRAUD_B,
                                op0=mybir.AluOpType.mult,
                                op1=mybir.AluOpType.add,
                            )
                        else:
                            nc.scalar.activation(
                                pt[:], sc[:],
                                mybir.ActivationFunctionType.Exp,
                                bias=0.0, scale=SCALE,
                            )
                        if it == 0 and hp == 0 and jc == 0:
                            emit_hsT_tail()
                        if pending and jc == 5:
                            norm_pe(pending.pop(0), qk0, ap)
                        for j in vtodo.get(jc, ()):
                            emit_vproj(j, ap)
                        for _ in range(2):
                            if bgq:
                                bgq.popleft()()
                        nc.tensor.matmul(
                            pv0[:],
                            v_sb[jc][:, h0 * VST : (h0 + 1) * VST],
                            pt[:, 0:512],
                            start=(jc == 0),
                            stop=(jc == NJC - 1),
                        )
                        nc.tensor.matmul(
                            pv1[:],
                            v_sb[jc][:, h1 * VST : (h1 + 1) * VST],
                            pt[:, 512:1024],
                            start=(jc == 0),
                            stop=(jc == NJC - 1),
                        )
                    pending.append(norm_dve(hp, pv0, pv1, isl,
                                            tail=(it == 1 and hp == CCH - 1)))
            while bgq:
                bgq.popleft()()
            # tail: the final pair's attn2[4] gates only the r=4 matmul of
            # each output projection -- accumulate r=0..3 for one ec (one
            # pp PSUM slot; the other must stay free for the norm's R)
            # underneath the reciprocal, then finish
            tail_ops = [oproj_ops(ec, 1) for ec in range(CCH)]
            for f in tail_ops[0][0:4]:
                f()
            for st in pending:
                norm_pe(st, None, ap)
            for ec in range(CCH):
                for f in tail_ops[ec][4:] if ec < 1 else tail_ops[ec]:
                    f()

    _spill_matmul_waits(nc)
    return nc


# walrus embedded-sync-wait capacity per BIR opcode.  Matmult holds a
# single wait; excess waits hoist onto the paired Ldweights (in-order
# issue on PE makes that equivalent).  Other compute ops spill onto
# EventSemaphore carrier instructions inserted just before them on the
# same engine.  DMACopy / Drain / EventSemaphore handle many waits
# natively (bacc emits such itself) and are left alone.
_WAIT_CAPS = {
    "InstMatmult": 1,
    "InstLdweights": 1,
    "InstActivation": 1,
    "InstReciprocal": 1,
    "InstTensorTensor": 1,
    "InstTensorCopy": 1,
    "InstTensorScalarPtr": 1,
    "InstTensorReduce": 1,
    "InstMemset": 1,
    "InstDMACopy": 1,
    "InstDrain": 1,
    "InstCustomDveAnt": 1,
}
_ES_CAP = 2  # waits per EventSemaphore carrier (walrus: <=2 waits, <=1 update)


def _spill_matmul_waits(nc: bass.Bass) -> None:
    spill_id = [0]

    def carriers(excess, engine):
        out = []
        for i in range(0, len(excess), _ES_CAP):
            es = mybir.InstEventSemaphore(
                name=f"wait-spill-{spill_id[0]}", ins=[], outs=[]
            )
            spill_id[0] += 1
            es.engine = engine
            es.sync_info = mybir.SyncInfo(
                on_wait=excess[i : i + _ES_CAP], on_update=[]
            )
            out.append(es)
        return out

    for f in nc.m.functions:
        for blk in f.blocks:
            insts = blk.instructions
            i = 0
            while i < len(insts):
                inst = insts[i]
                tn = type(inst).__name__
                cap = _WAIT_CAPS.get(tn)
                si = inst.sync_info
                if cap is None or si is None or len(si.on_wait) <= cap:
                    i += 1
                    continue
                w = list(si.on_wait)
                if tn == "InstMatmult" and cap == 1:
                    # Keep the latest-satisfied dependency (the ACT-produced
                    # operand, e.g. probs from exp) embedded on the matmul and
                    # hoist early ones onto the Ldweights: a wait on the LDW
                    # blocks its background prefetch and serializes ~50ns of
                    # weight-load into every PV matmul.
                    acts = [x for x in w if "Activation" in (x.ant_name or "")]
                    if acts:
                        keep = [acts[-1]]
                        excess = [x for x in w if x is not acts[-1]]
                    else:
                        keep, excess = w[-cap:], w[:-cap]
                else:
                    keep, excess = w[-cap:], w[:-cap]
                prev = insts[i - 1] if i > 0 else None
                if (
                    tn == "InstMatmult"
                    and prev is not None
                    and type(prev).__name__ == "InstLdweights"
                    and len(((prev.sync_info and prev.sync_info.on_wait) or []))
                    + len(excess) <= 1
                ):
                    psi = prev.sync_info
                    pw = list(psi.on_wait) if psi is not None else []
                    pu = list(psi.on_update) if psi is not None else []
                    prev.sync_info = mybir.SyncInfo(on_wait=pw + excess, on_update=pu)
                else:
                    new = carriers(excess, inst.engine)
                    insts[i:i] = new
                    i += len(new)
                inst.sync_info = mybir.SyncInfo(
                    on_wait=keep, on_update=list(si.on_update)
                )
                i += 1


_CACHED_NC = None


def get_nc() -> bass.Bass:
    global _CACHED_NC
    if _CACHED_NC is None:
        _CACHED_NC = build_nc()
    return _CACHED_NC


def make_in_maps(hidden_states, Wq, Wk, Wv, Wo, b_out):
    hs = np.asarray(hidden_states, dtype=np.float32)
    bf = ml_dtypes.bfloat16
    wqT = np.ascontiguousarray(np.asarray(Wq, np.float32).T).astype(bf)
    wkT = np.ascontiguousarray(np.asarray(Wk, np.float32).T).astype(bf)
    wvT = np.ascontiguousarray(np.asarray(Wv, np.float32).T).astype(bf)
    woT = np.ascontiguousarray(np.asarray(Wo, np.float32).T).astype(bf)
    bias = np.asarray(b_out, np.float32).reshape(C, 1)
    in_maps = []
    for c in range(NCORES):
        b, g = divmod(c, GROUP)
        i0 = g * SQ
        hsTb = hs[b].T  # [C, S]
        in_maps.append(
            {
                "hsT": np.ascontiguousarray(np.roll(hsTb, -i0, axis=1)).astype(bf),
                "res": np.ascontiguousarray(hsTb[:, i0 : i0 + SQ]) + bias,
                "wqT": wqT,
                "wkT": wkT,
                "wvT": wvT,
                "woT": woT,
            }
        )
    return in_maps


def assemble(results) -> np.ndarray:
    y = np.empty((B, S, C), np.float32)
    for c in range(NCORES):
        b, g = divmod(c, GROUP)
        i0 = g * SQ
        y[b, i0 : i0 + SQ, :] = np.asarray(results[c]["out"], np.float32).T
    return y


def kernel(**inputs) -> np.ndarray:
    from concourse.bass_utils import run_bass_kernel_spmd

    nc = get_nc()
    in_maps = make_in_maps(**inputs)
    res = run_bass_kernel_spmd(nc, in_maps, list(range(NCORES)))
    return assemble(res.results)


if __name__ == "__main__":
    import reference

    inputs = {k: np.asarray(v) for k, v in reference.setup_inputs().items()}
    got = kernel(**inputs)
    want = np.asarray(reference.reference(**inputs))
    err = np.linalg.norm(got - want) / np.linalg.norm(want)
    print("Relative error:", err)

